# revision 41
# baseline (speedup 1.0000x reference)
"""Trainium2 Bass kernel for nn_Attention (LN + QKV + 8-head attention + out-proj).

Sharding: data-parallel over the 16 (b,p) groups -> 2 groups per core, weights
replicated, no collectives.

Per-core dataflow (all matmul compute in bf16, f32 accumulation):
  x [2,1024,512] f32
  LN per i-tile (bn_stats/bn_aggr) -> xhat bf16 -> PE-transpose -> xhatT [d,i]
  QKV: qkT [e,i] = w1.T-chunks @ xhatT  (q,k in head-transposed layout)
       V   [j,c] = xhatT-chunks @ w1_v  (natural layout)
  Attention per head:
       ST [j,i] = kT.T @ qT   (row-packed head pairs, K=64)
       PT = exp(ST/8) on ScalarE (bf16)
       aot[0:64] = V.T @ PT   accumulated over j-chunks (PSUM)
       aot[64]   = ones.T @ PT  (softmax denominator row, concurrent col-strip)
       recip = 1/aot[64] (DVE), bc = ones_col @ recip (K=1 matmul broadcast)
       aoT = aot[0:64] * bc   (normalized, bf16)
  final [i,dd] = aoT-chunks.T @ w2, DMA out.
"""
import sys
import os

sys.path.insert(0, "/opt/trn_rl_repo")

import numpy as np
import ml_dtypes
from contextlib import ExitStack

import concourse.bass as bass
import concourse.bacc as bacc

# Steer Bacc's activation-table selection to the set containing BOTH exp and
# ln ("natural_log_exp_and_others") so the kernel runs with zero mid-stream
# ACT_TABLE_LOAD swaps. Table ids stay aligned with act_info.json since only
# membership is filtered, not order.
if not getattr(bacc, "_act_tbl_patched", False):
    _orig_gat = bacc.get_activation_tables

    def _gat_one_set(arch):
        tables = {k: set(v) for k, v in _orig_gat(arch).items()}
        AFT = mybir.ActivationFunctionType
        for name, funcs in tables.items():
            if name != "natural_log_exp_and_others":
                funcs.discard(AFT.Exp)
                funcs.discard(AFT.Ln)
        return tables

    bacc.get_activation_tables = _gat_one_set
    bacc._act_tbl_patched = True
import concourse.mybir as mybir
from concourse import tile
from concourse.masks import make_identity
from concourse.bass_utils import run_bass_kernel_spmd

F32 = mybir.dt.float32
BF16 = mybir.dt.bfloat16
AF = mybir.ActivationFunctionType
ALU = mybir.AluOpType
BF = ml_dtypes.bfloat16

# problem constants (hardcoded per harness rules)
B, P, N, D = 2, 8, 1024, 512
_last_res = None
HEADS, HD = 8, 64
INNER = HEADS * HD            # 512
E = 3 * INNER                 # 1536
EPS = 1e-5
SCALE = HD ** -0.5            # 0.125
NT = N // 128                 # 8 i-tiles
DC = D // 128                 # 4 d-chunks
G_PER_CORE = 2                # 16 groups / 8 cores
N_CORES = 8


def build_graph(use_b1: bool):
    nc = bacc.Bacc("TRN2", target_bir_lowering=False, debug=False)

    x = nc.declare_dram_parameter("x", [G_PER_CORE, N, D], F32, isOutput=False)
    w1 = nc.declare_dram_parameter("w1", [D, E], BF16, isOutput=False)
    w2 = nc.declare_dram_parameter("w2", [INNER, D], BF16, isOutput=False)
    if use_b1:
        b1 = nc.declare_dram_parameter("b1", [128, E // 128], F32, isOutput=False)
        b1v = nc.declare_dram_parameter("b1v", [1, INNER], BF16, isOutput=False)
    out = nc.declare_dram_parameter("out", [G_PER_CORE, N, D], F32, isOutput=True)
    rscratch = nc.dram_tensor("rscratch", [G_PER_CORE, 2, 4, N], BF16)

    with tile.TileContext(nc) as tc, ExitStack() as ctx:
        const = ctx.enter_context(tc.tile_pool(name="const", bufs=1))
        ln_pool = ctx.enter_context(tc.tile_pool(name="ln", bufs=5))
        xt_pool = ctx.enter_context(tc.tile_pool(name="xt", bufs=2))
        qkv_pool = ctx.enter_context(tc.tile_pool(name="qkv", bufs=2))
        pt_pool = ctx.enter_context(tc.tile_pool(name="pt", bufs=12))
        ao_pool = ctx.enter_context(tc.tile_pool(name="ao", bufs=2))
        aux_pool = ctx.enter_context(tc.tile_pool(name="aux", bufs=2))
        out_pool = ctx.enter_context(tc.tile_pool(name="outp", bufs=4))
        # PSUM budget (8 banks): st 2x2 + ao 1x2 + qkv 2x1 = 8
        ps_st = ctx.enter_context(tc.tile_pool(name="ps_st", bufs=2, space="PSUM"))
        ps_ao = ctx.enter_context(tc.tile_pool(name="ps_ao", bufs=1, space="PSUM"))
        ps_mm = ctx.enter_context(tc.tile_pool(name="ps_mm", bufs=2, space="PSUM"))

        # ---- constants / weights ----
        w1_sb = const.tile([128, DC, E], BF16)
        nc.gpsimd.dma_start(out=w1_sb,
                            in_=w1.rearrange("(dc p) e -> p dc e", p=128))
        w2_sb = const.tile([128, DC, D], BF16)
        nc.gpsimd.dma_start(out=w2_sb,
                            in_=w2.rearrange("(kc p) d -> p kc d", p=128))
        ident = const.tile([128, 128], BF16)
        make_identity(nc, ident)
        ones_row = const.tile([1, 128], BF16)
        nc.vector.memset(ones_row, 1.0)
        eps_sb = const.tile([128, 1], F32)
        nc.vector.memset(eps_sb, EPS)
        if use_b1:
            b1_sb = const.tile([128, E // 128], F32)
            nc.sync.dma_start(out=b1_sb, in_=b1[:, :])
            b1v_sb = const.tile([1, INNER], BF16)
            nc.sync.dma_start(out=b1v_sb, in_=b1v[:, :])

        for g in range(G_PER_CORE):
            # ---------------- LayerNorm + transpose ----------------
            xhatT = xt_pool.tile([128, DC, N], BF16, tag="xhatT")
            for t in range(NT):
                x_t = ln_pool.tile([128, D], F32, tag="x_t")
                nc.sync.dma_start(out=x_t, in_=x[g, 128 * t:128 * (t + 1), :])
                stats = ln_pool.tile([128, 6], F32, tag="stats")
                nc.vector.bn_stats(out=stats, in_=x_t)
                mv = ln_pool.tile([128, 2], F32, tag="mv")
                nc.vector.bn_aggr(out=mv, in_=stats)
                # rstd = exp(-0.5*ln(var+eps)) on ScalarE (cheap; avoids the
                # very expensive DVE reciprocal)
                lnv = ln_pool.tile([128, 1], F32, tag="lnv")
                nc.scalar.activation(out=lnv, in_=mv[:, 1:2], func=AF.Ln,
                                     bias=eps_sb, scale=1.0)
                rstd = ln_pool.tile([128, 1], F32, tag="rstd")
                nc.scalar.activation(out=rstd, in_=lnv, func=AF.Exp, scale=-0.5)
                xhat = ln_pool.tile([128, D], BF16, tag="xhat")
                nc.vector.tensor_scalar(out=xhat, in0=x_t, scalar1=mv[:, 0:1],
                                        scalar2=rstd, op0=ALU.subtract,
                                        op1=ALU.mult)
                tp = ps_mm.tile([128, 512], BF16, tag="mm")
                for dc in range(DC):
                    nc.tensor.transpose(tp[:, 128 * dc:128 * (dc + 1)],
                                        xhat[:, 128 * dc:128 * (dc + 1)], ident)
                nc.vector.tensor_copy(
                    out=xhatT[:, :, 128 * t:128 * (t + 1)],
                    in_=tp.rearrange("p (b c) -> p b c", b=DC))

            # ---------------- QKV projections ----------------
            # q,k in transposed layout: qkT[e, i], e-chunks 0..7 (4 q + 4 k)
            qkT = qkv_pool.tile([128, 8, N], BF16, tag="qkT")
            for c in range(8):
                accs = [ps_mm.tile([128, 512], F32, tag="mm",
                                   name=f"qk_g{g}_c{c}_{ic}") for ic in range(2)]
                for dc in range(DC):
                    for ic in range(2):
                        nc.tensor.matmul(
                            accs[ic], w1_sb[:, dc, 128 * c:128 * (c + 1)],
                            xhatT[:, dc, 512 * ic:512 * (ic + 1)],
                            start=(dc == 0), stop=(dc == DC - 1))
                for ic in range(2):
                    dst = qkT[:, c, 512 * ic:512 * (ic + 1)]
                    if use_b1:
                        nc.vector.tensor_scalar(out=dst, in0=accs[ic],
                                                scalar1=b1_sb[:, c:c + 1],
                                                scalar2=None, op0=ALU.add)
                    else:
                        nc.scalar.copy(out=dst, in_=accs[ic])
            # V in natural layout with a ones column per head:
            # v_sb[:, jt, h, 0:64] = V, v_sb[:, jt, h, 64] = 1.0 so the PV
            # matmul computes the softmax denominator in output row 64.
            v_sb = qkv_pool.tile([128, NT, HEADS, HD + 1], BF16, tag="v_sb")
            for t in range(NT):
                for h in range(HEADS):
                    nc.vector.memset(v_sb[:, t, h, HD:HD + 1], 1.0)
            for t in range(NT):
                acc = ps_mm.tile([128, 512], F32, tag="mm")
                for dc in range(DC):
                    nc.tensor.matmul(
                        acc, xhatT[:, dc, 128 * t:128 * (t + 1)],
                        w1_sb[:, dc, 2 * INNER:3 * INNER],
                        start=(dc == 0), stop=(dc == DC - 1 and not use_b1))
                if use_b1:
                    nc.tensor.matmul(acc, ones_row, b1v_sb,
                                     start=False, stop=True)
                nc.scalar.copy(
                    out=v_sb[:, t, :, 0:HD],
                    in_=acc.rearrange("p (h c) -> p h c", h=HEADS))

            # ---------------- attention ----------------
            aoT = ao_pool.tile([128, HEADS // 2, N], BF16, tag="aoT")
            aot_raw = {}
            for h in range(HEADS):
                hp, rlo = h // 2, 64 * (h % 2)
                aot = ps_ao.tile([128, N], F32, tag="ao", name=f"aot_g{g}_h{h}")
                for jt in range(NT):
                    st = ps_st.tile([128, N], F32, tag="st",
                                    name=f"st_g{g}_h{h}_j{jt}")
                    for ic in range(2):
                        nc.tensor.matmul(
                            st[:, 512 * ic:512 * (ic + 1)],
                            qkT[rlo:rlo + 64, 4 + hp, 128 * jt:128 * (jt + 1)],
                            qkT[rlo:rlo + 64, hp, 512 * ic:512 * (ic + 1)],
                            start=True, stop=True, tile_position=(rlo, 0))
                    pt = pt_pool.tile([128, N], BF16, tag="pt",
                                      name=f"pt_g{g}_h{h}_j{jt}")
                    nc.scalar.activation(out=pt, in_=st, func=AF.Exp,
                                         scale=SCALE)
                    for ic in range(2):
                        # M=65: V columns 0-63 + ones column -> row 64
                        # accumulates the softmax denominator.
                        nc.tensor.matmul(
                            aot[0:65, 512 * ic:512 * (ic + 1)],
                            v_sb[:, jt, h, :],
                            pt[:, 512 * ic:512 * (ic + 1)],
                            start=(jt == 0), stop=(jt == NT - 1))
                # stash raw PV output to SBUF so the single PSUM slot
                # recycles without waiting on the normalization chain; the
                # denominator rows of 4 heads collect at partitions
                # 0/32/64/96 of one tile for batched ln/exp.
                raw = ao_pool.tile([128, N], BF16, tag=f"raw{h}",
                                   name=f"aoraw_g{g}_h{h}", bufs=1)
                aot_raw[h] = raw
                nc.vector.tensor_copy(out=raw[0:65, :], in_=aot[0:65, :])
                if h == 0:
                    lr_t = {half: aux_pool.tile([97, N], F32,
                                                 tag=f"lr{half}",
                                                 name=f"lr_g{g}_{half}")
                            for half in range(2)}
                q = h % 4
                nc.vector.tensor_copy(out=lr_t[h // 4][32 * q:32 * q + 1, :],
                                      in_=raw[64:65, :])

            # ---- deferred normalization for all heads (overlaps next
            # group's LN/QKV phase); recip = exp(-ln(l)) batched 4 heads/op ----
            recip_all = {}
            for half in range(2):
                lnl = aux_pool.tile([97, N], F32, tag=f"lnl{half}")
                nc.scalar.activation(out=lnl, in_=lr_t[half], func=AF.Ln)
                r16 = aux_pool.tile([97, N], BF16, tag=f"r16{half}")
                nc.scalar.activation(out=r16, in_=lnl, func=AF.Exp, scale=-1.0)
                recip_all[half] = r16
                for q in range(4):
                    nc.sync.dma_start(out=rscratch[g, half, q, :],
                                      in_=r16[32 * q:32 * q + 1, :])
            for h in range(HEADS):
                hp, q = h // 2, h % 4
                r16 = recip_all[h // 4]
                # broadcast the recip row across 64 partitions via a DRAM
                # round-trip (partition-step-0 DMA read) -- keeps the PE out
                # of the normalization entirely
                bc_sb = aux_pool.tile([64, N], BF16, tag="bc_sb",
                                      name=f"bc_g{g}_h{h}")
                brd = rscratch[g, h // 4, q, :].partition_broadcast(64)
                nc.gpsimd.dma_start(out=bc_sb, in_=brd)
                ro = 64 * (h % 2)
                nc.vector.tensor_mul(aoT[ro:ro + 64, hp, :],
                                     aot_raw[h][0:64, :], bc_sb)

            # ---------------- final projection ----------------
            for t in range(NT):
                acc = ps_mm.tile([128, 512], F32, tag="mm")
                for kc in range(DC):
                    nc.tensor.matmul(
                        acc, aoT[:, kc, 128 * t:128 * (t + 1)],
                        w2_sb[:, kc, :],
                        start=(kc == 0), stop=(kc == DC - 1))
                o_t = out_pool.tile([128, D], F32, tag="o_t")
                nc.vector.tensor_copy(out=o_t, in_=acc)
                nc.sync.dma_start(out=out[g, 128 * t:128 * (t + 1), :], in_=o_t)

    nc.compile()
    return nc


def kernel(x, ln_w, ln_b, w_qkv, w_out):
    x = np.asarray(x, dtype=np.float32)
    ln_w = np.asarray(ln_w, dtype=np.float32)
    ln_b = np.asarray(ln_b, dtype=np.float32)
    w_qkv = np.asarray(w_qkv, dtype=np.float32)
    w_out = np.asarray(w_out, dtype=np.float32)

    # host-side weight folding (LN affine into QKV weights)
    w1 = (w_qkv * ln_w[None, :]).T.astype(BF)            # [D, E]
    b1 = (w_qkv @ ln_b).astype(np.float32)               # [E]
    w2 = w_out.T.astype(BF)                              # [INNER, D]
    use_b1 = bool(np.any(b1))

    nc = build_graph(use_b1)

    xg = x.reshape(B * P, N, D)
    in_maps = []
    for core in range(N_CORES):
        m = {
            "x": np.ascontiguousarray(xg[G_PER_CORE * core:G_PER_CORE * (core + 1)]),
            "w1": w1,
            "w2": w2,
        }
        if use_b1:
            m["b1"] = b1.reshape(E // 128, 128).T.astype(np.float32).copy()
            m["b1v"] = b1[2 * INNER:].reshape(1, INNER).astype(BF)
        in_maps.append(m)

    trace = bool(int(os.environ.get("KERNEL_TRACE", "0")))
    if trace:
        try:
            import ntff_shim
            ntff_shim.install()
        except Exception as e:
            print(f"ntff shim unavailable: {e}")
            trace = False
    res = run_bass_kernel_spmd(nc, in_maps, list(range(N_CORES)), trace=trace,
                               tmpdir=os.environ.get("KERNEL_TRACE_DIR"))
    global _last_res
    _last_res = res
    if res.exec_time_ns is not None:
        print(f"HW exec time: {res.exec_time_ns} ns")
    out = np.concatenate([r["out"] for r in res.results], axis=0)
    return out.reshape(B, P, N, D)


# revision 42
# speedup vs baseline: 1.0301x; 1.0301x over previous
"""Trainium2 Bass kernel for nn_Attention (LN + QKV + 8-head attention + out-proj).

Sharding: data-parallel over the 16 (b,p) groups -> 2 groups per core, weights
replicated, no collectives.

Per-core dataflow (all matmul compute in bf16, f32 accumulation):
  x [2,1024,512] f32
  LN per i-tile (bn_stats/bn_aggr) -> xhat bf16 -> PE-transpose -> xhatT [d,i]
  QKV: qkT [e,i] = w1.T-chunks @ xhatT  (q,k in head-transposed layout)
       V   [j,c] = xhatT-chunks @ w1_v  (natural layout)
  Attention per head:
       ST [j,i] = kT.T @ qT   (row-packed head pairs, K=64)
       PT = exp(ST/8) on ScalarE (bf16)
       aot[0:64] = V.T @ PT   accumulated over j-chunks (PSUM)
       aot[64]   = ones.T @ PT  (softmax denominator row, concurrent col-strip)
       recip = 1/aot[64] (DVE), bc = ones_col @ recip (K=1 matmul broadcast)
       aoT = aot[0:64] * bc   (normalized, bf16)
  final [i,dd] = aoT-chunks.T @ w2, DMA out.
"""
import sys
import os

sys.path.insert(0, "/opt/trn_rl_repo")

import numpy as np
import ml_dtypes
from contextlib import ExitStack

import concourse.bass as bass
import concourse.bacc as bacc

# Steer Bacc's activation-table selection to the set containing BOTH exp and
# ln ("natural_log_exp_and_others") so the kernel runs with zero mid-stream
# ACT_TABLE_LOAD swaps. Table ids stay aligned with act_info.json since only
# membership is filtered, not order.
if not getattr(bacc, "_act_tbl_patched", False):
    _orig_gat = bacc.get_activation_tables

    def _gat_one_set(arch):
        tables = {k: set(v) for k, v in _orig_gat(arch).items()}
        AFT = mybir.ActivationFunctionType
        for name, funcs in tables.items():
            if name != "natural_log_exp_and_others":
                funcs.discard(AFT.Exp)
                funcs.discard(AFT.Ln)
        return tables

    bacc.get_activation_tables = _gat_one_set
    bacc._act_tbl_patched = True
import concourse.mybir as mybir
from concourse import tile
from concourse.masks import make_identity
from concourse.bass_utils import run_bass_kernel_spmd

F32 = mybir.dt.float32
BF16 = mybir.dt.bfloat16
AF = mybir.ActivationFunctionType
ALU = mybir.AluOpType
BF = ml_dtypes.bfloat16

# problem constants (hardcoded per harness rules)
B, P, N, D = 2, 8, 1024, 512
_last_res = None
HEADS, HD = 8, 64
INNER = HEADS * HD            # 512
E = 3 * INNER                 # 1536
EPS = 1e-5
SCALE = HD ** -0.5            # 0.125
NT = N // 128                 # 8 i-tiles
DC = D // 128                 # 4 d-chunks
G_PER_CORE = 2                # 16 groups / 8 cores
N_CORES = 8


def build_graph(use_b1: bool):
    nc = bacc.Bacc("TRN2", target_bir_lowering=False, debug=False)

    x = nc.declare_dram_parameter("x", [G_PER_CORE, N, D], F32, isOutput=False)
    w1 = nc.declare_dram_parameter("w1", [D, E], BF16, isOutput=False)
    w2 = nc.declare_dram_parameter("w2", [INNER, D], BF16, isOutput=False)
    if use_b1:
        b1 = nc.declare_dram_parameter("b1", [128, E // 128], F32, isOutput=False)
        b1v = nc.declare_dram_parameter("b1v", [1, INNER], BF16, isOutput=False)
    out = nc.declare_dram_parameter("out", [G_PER_CORE, N, D], F32, isOutput=True)
    rscratch = nc.dram_tensor("rscratch", [G_PER_CORE, 2, 4, N], BF16)

    with tile.TileContext(nc) as tc, ExitStack() as ctx:
        const = ctx.enter_context(tc.tile_pool(name="const", bufs=1))
        ln_pool = ctx.enter_context(tc.tile_pool(name="ln", bufs=5))
        xt_pool = ctx.enter_context(tc.tile_pool(name="xt", bufs=2))
        qkv_pool = ctx.enter_context(tc.tile_pool(name="qkv", bufs=2))
        pt_pool = ctx.enter_context(tc.tile_pool(name="pt", bufs=12))
        ao_pool = ctx.enter_context(tc.tile_pool(name="ao", bufs=2))
        aux_pool = ctx.enter_context(tc.tile_pool(name="aux", bufs=2))
        out_pool = ctx.enter_context(tc.tile_pool(name="outp", bufs=4))
        # PSUM budget (8 banks): st 2x2 + ao 1x2 + qkv 2x1 = 8
        ps_st = ctx.enter_context(tc.tile_pool(name="ps_st", bufs=2, space="PSUM"))
        ps_ao = ctx.enter_context(tc.tile_pool(name="ps_ao", bufs=1, space="PSUM"))
        ps_mm = ctx.enter_context(tc.tile_pool(name="ps_mm", bufs=2, space="PSUM"))

        # ---- constants / weights ----
        w1_sb = const.tile([128, DC, E], BF16)
        nc.sync.dma_start(out=w1_sb, in_=w1.rearrange("(dc p) e -> p dc e", p=128))
        w2_sb = const.tile([128, DC, D], BF16)
        nc.sync.dma_start(out=w2_sb, in_=w2.rearrange("(kc p) d -> p kc d", p=128))
        ident = const.tile([128, 128], BF16)
        make_identity(nc, ident)
        ones_row = const.tile([1, 128], BF16)
        nc.vector.memset(ones_row, 1.0)
        eps_sb = const.tile([128, 1], F32)
        nc.vector.memset(eps_sb, EPS)
        if use_b1:
            b1_sb = const.tile([128, E // 128], F32)
            nc.sync.dma_start(out=b1_sb, in_=b1[:, :])
            b1v_sb = const.tile([1, INNER], BF16)
            nc.sync.dma_start(out=b1v_sb, in_=b1v[:, :])

        for g in range(G_PER_CORE):
            # ---------------- LayerNorm + transpose ----------------
            xhatT = xt_pool.tile([128, DC, N], BF16, tag="xhatT")
            for t in range(NT):
                x_t = ln_pool.tile([128, D], F32, tag="x_t")
                nc.sync.dma_start(out=x_t, in_=x[g, 128 * t:128 * (t + 1), :])
                stats = ln_pool.tile([128, 6], F32, tag="stats")
                nc.vector.bn_stats(out=stats, in_=x_t)
                mv = ln_pool.tile([128, 2], F32, tag="mv")
                nc.vector.bn_aggr(out=mv, in_=stats)
                # rstd = exp(-0.5*ln(var+eps)) on ScalarE (cheap; avoids the
                # very expensive DVE reciprocal)
                lnv = ln_pool.tile([128, 1], F32, tag="lnv")
                nc.scalar.activation(out=lnv, in_=mv[:, 1:2], func=AF.Ln,
                                     bias=eps_sb, scale=1.0)
                rstd = ln_pool.tile([128, 1], F32, tag="rstd")
                nc.scalar.activation(out=rstd, in_=lnv, func=AF.Exp, scale=-0.5)
                xhat = ln_pool.tile([128, D], BF16, tag="xhat")
                nc.vector.tensor_scalar(out=xhat, in0=x_t, scalar1=mv[:, 0:1],
                                        scalar2=rstd, op0=ALU.subtract,
                                        op1=ALU.mult)
                tp = ps_mm.tile([128, 512], BF16, tag="mm")
                for dc in range(DC):
                    nc.tensor.transpose(tp[:, 128 * dc:128 * (dc + 1)],
                                        xhat[:, 128 * dc:128 * (dc + 1)], ident)
                nc.vector.tensor_copy(
                    out=xhatT[:, :, 128 * t:128 * (t + 1)],
                    in_=tp.rearrange("p (b c) -> p b c", b=DC))

            # ---------------- QKV projections ----------------
            # q,k in transposed layout: qkT[e, i], e-chunks 0..7 (4 q + 4 k)
            qkT = qkv_pool.tile([128, 8, N], BF16, tag="qkT")
            for c in range(8):
                accs = [ps_mm.tile([128, 512], F32, tag="mm",
                                   name=f"qk_g{g}_c{c}_{ic}") for ic in range(2)]
                for dc in range(DC):
                    for ic in range(2):
                        nc.tensor.matmul(
                            accs[ic], w1_sb[:, dc, 128 * c:128 * (c + 1)],
                            xhatT[:, dc, 512 * ic:512 * (ic + 1)],
                            start=(dc == 0), stop=(dc == DC - 1))
                for ic in range(2):
                    dst = qkT[:, c, 512 * ic:512 * (ic + 1)]
                    if use_b1:
                        nc.vector.tensor_scalar(out=dst, in0=accs[ic],
                                                scalar1=b1_sb[:, c:c + 1],
                                                scalar2=None, op0=ALU.add)
                    else:
                        nc.scalar.copy(out=dst, in_=accs[ic])
            # V in natural layout with a ones column per head:
            # v_sb[:, jt, h, 0:64] = V, v_sb[:, jt, h, 64] = 1.0 so the PV
            # matmul computes the softmax denominator in output row 64.
            v_sb = qkv_pool.tile([128, NT, HEADS, HD + 1], BF16, tag="v_sb")
            for t in range(NT):
                for h in range(HEADS):
                    nc.vector.memset(v_sb[:, t, h, HD:HD + 1], 1.0)
            for t in range(NT):
                acc = ps_mm.tile([128, 512], F32, tag="mm")
                for dc in range(DC):
                    nc.tensor.matmul(
                        acc, xhatT[:, dc, 128 * t:128 * (t + 1)],
                        w1_sb[:, dc, 2 * INNER:3 * INNER],
                        start=(dc == 0), stop=(dc == DC - 1 and not use_b1))
                if use_b1:
                    nc.tensor.matmul(acc, ones_row, b1v_sb,
                                     start=False, stop=True)
                nc.scalar.copy(
                    out=v_sb[:, t, :, 0:HD],
                    in_=acc.rearrange("p (h c) -> p h c", h=HEADS))

            # ---------------- attention ----------------
            aoT = ao_pool.tile([128, HEADS // 2, N], BF16, tag="aoT")
            aot_raw = {}
            for h in range(HEADS):
                hp, rlo = h // 2, 64 * (h % 2)
                aot = ps_ao.tile([128, N], F32, tag="ao", name=f"aot_g{g}_h{h}")
                for jt in range(NT):
                    st = ps_st.tile([128, N], F32, tag="st",
                                    name=f"st_g{g}_h{h}_j{jt}")
                    for ic in range(2):
                        nc.tensor.matmul(
                            st[:, 512 * ic:512 * (ic + 1)],
                            qkT[rlo:rlo + 64, 4 + hp, 128 * jt:128 * (jt + 1)],
                            qkT[rlo:rlo + 64, hp, 512 * ic:512 * (ic + 1)],
                            start=True, stop=True, tile_position=(rlo, 0))
                    pt = pt_pool.tile([128, N], BF16, tag="pt",
                                      name=f"pt_g{g}_h{h}_j{jt}")
                    nc.scalar.activation(out=pt, in_=st, func=AF.Exp,
                                         scale=SCALE)
                    for ic in range(2):
                        # M=65: V columns 0-63 + ones column -> row 64
                        # accumulates the softmax denominator.
                        nc.tensor.matmul(
                            aot[0:65, 512 * ic:512 * (ic + 1)],
                            v_sb[:, jt, h, :],
                            pt[:, 512 * ic:512 * (ic + 1)],
                            start=(jt == 0), stop=(jt == NT - 1))
                # stash raw PV output to SBUF so the single PSUM slot
                # recycles without waiting on the normalization chain; the
                # denominator rows of 4 heads collect at partitions
                # 0/32/64/96 of one tile for batched ln/exp.
                raw = ao_pool.tile([128, N], BF16, tag=f"raw{h}",
                                   name=f"aoraw_g{g}_h{h}", bufs=1)
                aot_raw[h] = raw
                nc.vector.tensor_copy(out=raw[0:65, :], in_=aot[0:65, :])
                if h == 0:
                    lr_t = {half: aux_pool.tile([97, N], F32,
                                                 tag=f"lr{half}",
                                                 name=f"lr_g{g}_{half}")
                            for half in range(2)}
                q = h % 4
                nc.vector.tensor_copy(out=lr_t[h // 4][32 * q:32 * q + 1, :],
                                      in_=raw[64:65, :])

            # ---- deferred normalization for all heads (overlaps next
            # group's LN/QKV phase); recip = exp(-ln(l)) batched 4 heads/op ----
            recip_all = {}
            for half in range(2):
                lnl = aux_pool.tile([97, N], F32, tag=f"lnl{half}")
                nc.scalar.activation(out=lnl, in_=lr_t[half], func=AF.Ln)
                r16 = aux_pool.tile([97, N], BF16, tag=f"r16{half}")
                nc.scalar.activation(out=r16, in_=lnl, func=AF.Exp, scale=-1.0)
                recip_all[half] = r16
                for q in range(4):
                    nc.sync.dma_start(out=rscratch[g, half, q, :],
                                      in_=r16[32 * q:32 * q + 1, :])
            for h in range(HEADS):
                hp, q = h // 2, h % 4
                r16 = recip_all[h // 4]
                # broadcast the recip row across 64 partitions via a DRAM
                # round-trip (partition-step-0 DMA read) -- keeps the PE out
                # of the normalization entirely
                bc_sb = aux_pool.tile([64, N], BF16, tag="bc_sb",
                                      name=f"bc_g{g}_h{h}")
                brd = rscratch[g, h // 4, q, :].partition_broadcast(64)
                nc.gpsimd.dma_start(out=bc_sb, in_=brd)
                ro = 64 * (h % 2)
                nc.vector.tensor_mul(aoT[ro:ro + 64, hp, :],
                                     aot_raw[h][0:64, :], bc_sb)

            # ---------------- final projection ----------------
            for t in range(NT):
                acc = ps_mm.tile([128, 512], F32, tag="mm")
                for kc in range(DC):
                    nc.tensor.matmul(
                        acc, aoT[:, kc, 128 * t:128 * (t + 1)],
                        w2_sb[:, kc, :],
                        start=(kc == 0), stop=(kc == DC - 1))
                o_t = out_pool.tile([128, D], F32, tag="o_t")
                nc.vector.tensor_copy(out=o_t, in_=acc)
                nc.sync.dma_start(out=out[g, 128 * t:128 * (t + 1), :], in_=o_t)

    nc.compile()
    return nc


def kernel(x, ln_w, ln_b, w_qkv, w_out):
    x = np.asarray(x, dtype=np.float32)
    ln_w = np.asarray(ln_w, dtype=np.float32)
    ln_b = np.asarray(ln_b, dtype=np.float32)
    w_qkv = np.asarray(w_qkv, dtype=np.float32)
    w_out = np.asarray(w_out, dtype=np.float32)

    # host-side weight folding (LN affine into QKV weights)
    w1 = (w_qkv * ln_w[None, :]).T.astype(BF)            # [D, E]
    b1 = (w_qkv @ ln_b).astype(np.float32)               # [E]
    w2 = w_out.T.astype(BF)                              # [INNER, D]
    use_b1 = bool(np.any(b1))

    nc = build_graph(use_b1)

    xg = x.reshape(B * P, N, D)
    in_maps = []
    for core in range(N_CORES):
        m = {
            "x": np.ascontiguousarray(xg[G_PER_CORE * core:G_PER_CORE * (core + 1)]),
            "w1": w1,
            "w2": w2,
        }
        if use_b1:
            m["b1"] = b1.reshape(E // 128, 128).T.astype(np.float32).copy()
            m["b1v"] = b1[2 * INNER:].reshape(1, INNER).astype(BF)
        in_maps.append(m)

    trace = bool(int(os.environ.get("KERNEL_TRACE", "0")))
    if trace:
        try:
            import ntff_shim
            ntff_shim.install()
        except Exception as e:
            print(f"ntff shim unavailable: {e}")
            trace = False
    res = run_bass_kernel_spmd(nc, in_maps, list(range(N_CORES)), trace=trace,
                               tmpdir=os.environ.get("KERNEL_TRACE_DIR"))
    global _last_res
    _last_res = res
    if res.exec_time_ns is not None:
        print(f"HW exec time: {res.exec_time_ns} ns")
    out = np.concatenate([r["out"] for r in res.results], axis=0)
    return out.reshape(B, P, N, D)
